# revision 1
# baseline (speedup 1.0000x reference)
"""Trainium2 kernel for nn_LoRALinear (moe_routing).

Math: reference computes out = x @ W.T + einsum('bri,bro->bo', a, b) with
a = A_table[dom].reshape(B,R,IN), b = B_table[dom].reshape(B,R,OUT).
The einsum contracts i over `a` alone, so the LoRA term collapses to a
per-domain table:
    L[d, o] = sum_r (sum_i A_table[d].reshape(R,IN)[r,i]) * B_table[d].reshape(R,OUT)[r,o]
    out = x @ W.T + L[domain_id]

On device this is a single augmented matmul per batch row:
    out[m, :] = [x[m, :], onehot(dom[m])] @ [[W.T], [L]]
with contraction K = 1024 (8 chunks of 128) plus a K=64 one-hot chunk.
The one-hot rows select L rows exactly (0/1 are exact in bf16). The two
K=64 one-hot matmuls per m-tile are packed into disjoint PE row groups
(tile_position) so they run concurrently.

Sharding: data-parallel over batch across 8 cores; the augmented weight is
replicated.

Device layout: the host pre-transposes activations into chunk-major form
xa[p, mb, k, j] = xaT[k*128 + p, mb*MB + j] so each m-block is a single
contiguous-per-partition DMA covering all 9 K-chunks (chunk 8 carries the
one-hot rows duplicated into both half-partitions).
"""

import functools

import numpy as np

import concourse.mybir as mybir
import concourse.tile as tile
from concourse import bacc, bass_utils

B, D, R, ND = 16384, 1024, 8, 64
N_CORES = 8
BS = B // N_CORES            # 2048 batch rows per core
NKW = 8                      # K chunks of 128 for the dense W part
NK = NKW + 1                 # + one-hot chunk
MB = 512                     # batch rows per x block DMA
NMB = BS // MB               # 4 blocks
OH = 512                     # psum free dim (one bank)


@functools.lru_cache(maxsize=1)
def _build():
    nc = bacc.Bacc(None, target_bir_lowering=False, debug=False)
    bf16 = mybir.dt.bfloat16
    xa = nc.dram_tensor("xa", [128, NMB * NK * MB], bf16, kind="ExternalInput")
    wa = nc.dram_tensor("wa", [NKW * 128, D], bf16, kind="ExternalInput")
    # L table packed for row-group concurrency: rows 0:64 = L[:, 0:512],
    # rows 64:128 = L[:, 512:1024]
    w8 = nc.dram_tensor("w8", [128, OH], bf16, kind="ExternalInput")
    out = nc.dram_tensor("out", [BS, D], mybir.dt.float32, kind="ExternalOutput")

    with tile.TileContext(nc) as tc:
        with (
            tc.tile_pool(name="w", bufs=1) as wpool,
            tc.tile_pool(name="x", bufs=2) as xpool,
            tc.tile_pool(name="o", bufs=4) as opool,
            tc.tile_pool(name="ps", bufs=7, space="PSUM") as pspool,
            tc.tile_pool(name="dps", bufs=1, space="PSUM") as dpspool,
        ):
            # Warm the PE (HAM clock gate) with dummy matmuls on a scratch
            # tile while the first DMAs stream in; otherwise the first ~12
            # real matmuls run at half clock.
            scratch = wpool.tile([128, OH], bf16, tag="scratch")
            nc.gpsimd.memset(scratch[:], 0.0)
            dps = dpspool.tile([128, OH], mybir.dt.float32, tag="dps")
            for i in range(12):
                nc.tensor.matmul(
                    dps[:],
                    scratch[:, 0:128],
                    scratch[:],
                    start=(i == 0),
                    stop=(i == 11),
                )

            # x block 0 first so its transfer overlaps the W preload.
            xts = {}
            xt0 = xpool.tile([128, NK * MB], bf16, tag="x")
            nc.sync.dma_start(xt0[:], xa[:, 0 : NK * MB])
            xts[0] = xt0

            wts = []
            for k in range(NKW):
                wt = wpool.tile([128, D], bf16, tag=f"w{k}")
                nc.sync.dma_start(wt[:], wa[k * 128 : (k + 1) * 128, :])
                wts.append(wt)
            w8t = wpool.tile([128, OH], bf16, tag="w8")
            nc.sync.dma_start(w8t[:], w8[:, :])

            def xsl(xt, k, mt):
                return xt[:, k * MB + mt * 128 : k * MB + (mt + 1) * 128]

            def finish(xt, mt, pss, mb):
                """One-hot row-group-packed matmuls + psum copies + out DMA."""
                nc.tensor.matmul(
                    pss[0][:],
                    xt[0:64, NKW * MB + mt * 128 : NKW * MB + (mt + 1) * 128],
                    w8t[0:64, :],
                    start=False,
                    stop=True,
                    tile_position=(0, 0),
                )
                nc.tensor.matmul(
                    pss[1][:],
                    xt[64:128, NKW * MB + mt * 128 : NKW * MB + (mt + 1) * 128],
                    w8t[64:128, :],
                    start=False,
                    stop=True,
                    tile_position=(64, 0),
                )
                ot = opool.tile([128, D], mybir.dt.float32, tag="ot")
                nc.vector.tensor_copy(ot[:, 0:OH], pss[0][:])
                nc.scalar.copy(ot[:, OH : 2 * OH], pss[1][:])
                m0 = mb * MB + mt * 128
                nc.sync.dma_start(out[m0 : m0 + 128, :], ot[:])

            # First two m-tiles: k-interleaved across 4 psum groups so each
            # arriving W chunk immediately feeds 4 matmuls (keeps the PE fed
            # while W streams in).
            pss = {}
            for g in range(4):
                psg = pspool.tile([128, OH], mybir.dt.float32, tag="ps")
                pss[g] = psg
            for k in range(NKW):
                for g in range(4):
                    mt, oh = divmod(g, 2)
                    nc.tensor.matmul(
                        pss[g][:],
                        xsl(xt0, k, mt),
                        wts[k][:, oh * OH : (oh + 1) * OH],
                        start=(k == 0),
                        stop=False,
                    )
            finish(xt0, 0, (pss[0], pss[1]), 0)
            finish(xt0, 1, (pss[2], pss[3]), 0)

            for mb in range(NMB):
                if mb not in xts:
                    xtn = xpool.tile([128, NK * MB], bf16, tag="x")
                    nc.sync.dma_start(
                        xtn[:], xa[:, mb * NK * MB : (mb + 1) * NK * MB]
                    )
                    xts[mb] = xtn
                xt = xts[mb]
                for mt in range(MB // 128):
                    if mb == 0 and mt < 2:
                        continue  # handled by the k-interleaved prologue
                    ps0 = pspool.tile([128, OH], mybir.dt.float32, tag="ps")
                    ps1 = pspool.tile([128, OH], mybir.dt.float32, tag="ps")
                    for k in range(NKW):
                        nc.tensor.matmul(
                            ps0[:],
                            xsl(xt, k, mt),
                            wts[k][:, 0:OH],
                            start=(k == 0),
                            stop=False,
                        )
                    for k in range(NKW):
                        nc.tensor.matmul(
                            ps1[:],
                            xsl(xt, k, mt),
                            wts[k][:, OH : 2 * OH],
                            start=(k == 0),
                            stop=False,
                        )
                    finish(xt, mt, (ps0, ps1), mb)

    nc.compile()
    return nc


def _prepare(x, W, A_table, B_table, domain_id):
    import ml_dtypes

    bf16 = np.dtype(ml_dtypes.bfloat16)
    x = np.asarray(x, dtype=np.float32)
    W = np.asarray(W, dtype=np.float32)
    A = np.asarray(A_table, dtype=np.float64)
    Bt = np.asarray(B_table, dtype=np.float64)
    dom = np.asarray(domain_id).astype(np.int64)

    sA = A.reshape(ND, R, D).sum(axis=2)                        # [ND, R]
    L = np.einsum("dr,dro->do", sA, Bt.reshape(ND, R, D))       # [ND, D]
    Lb = L.astype(np.float32).astype(bf16)

    wa = np.ascontiguousarray(W.T.astype(bf16))                 # [D, D]
    w8 = np.empty((128, OH), dtype=bf16)
    w8[0:ND] = Lb[:, 0:OH]
    w8[ND : 2 * ND] = Lb[:, OH : 2 * OH]

    xT = np.ascontiguousarray(x.T).astype(bf16)                 # [D, B]
    onehotT = (
        np.arange(ND, dtype=np.int64)[:, None] == dom[None, :]
    ).astype(bf16)                                              # [ND, B]

    in_maps = []
    for c in range(N_CORES):
        sl = slice(c * BS, (c + 1) * BS)
        xaT_c = np.empty((NK * 128, BS), dtype=bf16)
        xaT_c[: NKW * 128] = xT[:, sl]
        xaT_c[NKW * 128 : NKW * 128 + ND] = onehotT[:, sl]
        xaT_c[NKW * 128 + ND :] = onehotT[:, sl]                # duplicate
        # chunk-major: xa[p, mb, k, j] = xaT_c[k*128 + p, mb*MB + j]
        xa_c = np.ascontiguousarray(
            xaT_c.reshape(NK, 128, NMB, MB).transpose(1, 2, 0, 3)
        ).reshape(128, NMB * NK * MB)
        in_maps.append({"xa": xa_c, "wa": wa, "w8": w8})
    return in_maps


def kernel(x, W, A_table, B_table, domain_id, _trace=False):
    in_maps = _prepare(x, W, A_table, B_table, domain_id)
    nc = _build()
    res = bass_utils.run_bass_kernel_spmd(
        nc, in_maps, core_ids=list(range(N_CORES)), trace=_trace
    )
    out = np.concatenate([res.results[c]["out"] for c in range(N_CORES)], axis=0)
    if _trace:
        kernel.last_results = res
    return out



# revision 3
# speedup vs baseline: 1.0754x; 1.0754x over previous
"""Trainium2 kernel for nn_LoRALinear (moe_routing).

Math: reference computes out = x @ W.T + einsum('bri,bro->bo', a, b) with
a = A_table[dom].reshape(B,R,IN), b = B_table[dom].reshape(B,R,OUT).
The einsum contracts i over `a` alone, so the LoRA term collapses to a
per-domain table:
    L[d, o] = sum_r (sum_i A_table[d].reshape(R,IN)[r,i]) * B_table[d].reshape(R,OUT)[r,o]
    out = x @ W.T + L[domain_id]

On device: pure dense matmul x @ W.T (K=1024 as 8 chunks of 128) in bf16;
the LoRA term is added during PSUM evacuation by the vector engine from a
host-gathered per-row table Lg = L[domain_id] streamed in as bf16.
Output is written in bf16 (rel-err budget 2e-2; bf16 rounding adds ~1e-3)
and upcast to f32 on the host.

Sharding: data-parallel over batch across 8 cores; W replicated.

Device layout: host pre-transposes activations into chunk-major form
xa[p, mb, k, j] = x.T[k*128 + p, mb*MB + j]. Block 0 is DMAed chunk by
chunk, interleaved with the 8 W-chunk DMAs, so the PE can start at ~1.2us
and is fed at the rate W arrives. Out-stores ride the ACT HWDGE ring so
they never queue behind input loads on the sync ring.
"""

import functools

import numpy as np

import concourse.mybir as mybir
import concourse.tile as tile
from concourse import bacc, bass_utils

B, D, R, ND = 16384, 1024, 8, 64
N_CORES = 8
BS = B // N_CORES            # 2048 batch rows per core
NK = 8                       # K chunks of 128
MB = 512                     # batch rows per xa block (4 m-tiles)
NMB = BS // MB               # 4 blocks
NMT = 4                      # m-tiles per block
OH = 512                     # psum free dim (one bank)


@functools.lru_cache(maxsize=1)
def _build():
    nc = bacc.Bacc(None, target_bir_lowering=False, debug=False)
    bf16 = mybir.dt.bfloat16
    f32 = mybir.dt.float32
    xa = nc.dram_tensor("xa", [128, NMB * NK * MB], bf16, kind="ExternalInput")
    wa = nc.dram_tensor("wa", [NK * 128, D], bf16, kind="ExternalInput")
    # Lg rows packed per m-tile: lga[p, mb, mt, o] = L[dom[(mb*4+mt)*128+p], o]
    lga = nc.dram_tensor("lga", [128, NMB * NMT * D], bf16, kind="ExternalInput")
    out = nc.dram_tensor("out", [BS, D], bf16, kind="ExternalOutput")

    with tile.TileContext(nc) as tc:
        with (
            tc.tile_pool(name="w", bufs=1) as wpool,
            tc.tile_pool(name="x", bufs=2) as xpool,
            tc.tile_pool(name="lg", bufs=2) as lgpool,
            tc.tile_pool(name="o", bufs=6) as opool,
            tc.tile_pool(name="ps", bufs=8, space="PSUM") as pspool,
        ):
            # Input DMAs on the sync ring, in need-order: xa chunk k of
            # block 0 interleaved with W chunk k, then lg block 0, then the
            # remaining blocks.
            xts = {}
            xt0 = xpool.tile([128, NK * MB], bf16, tag="x")
            wts = []
            for k in range(NK):
                nc.sync.dma_start(xt0[:, k * MB : (k + 1) * MB], xa[:, k * MB : (k + 1) * MB])
                wt = wpool.tile([128, D], bf16, tag=f"w{k}")
                nc.sync.dma_start(wt[:], wa[k * 128 : (k + 1) * 128, :])
                wts.append(wt)
            xts[0] = xt0
            lgts = {}
            lgt0 = lgpool.tile([128, NMT * D], bf16, tag="lg")
            nc.sync.dma_start(lgt0[:], lga[:, 0 : NMT * D])
            lgts[0] = lgt0

            # Warm the PE (HAM clock gate) with small dummy matmuls so the
            # clock is ramping while the first DMAs land. They write into a
            # psum slot that real accumulation later reclaims (start=True
            # clears the bank).
            scratch = wpool.tile([128, 256], bf16, tag="scratch")
            nc.vector.memset(scratch[:], 0.0)
            psw = pspool.tile([128, OH], f32, tag="ps")
            for i in range(12):
                nc.tensor.matmul(
                    psw[:, 0:128],
                    scratch[:, 0:128],
                    scratch[:, 128:256],
                    start=(i == 0),
                    stop=(i == 11),
                )

            def xsl(xt, k, mt):
                return xt[:, k * MB + mt * 128 : k * MB + (mt + 1) * 128]

            def finish(mb, mt, ps0, ps1, lgt):
                ot = opool.tile([128, D], bf16, tag="ot")
                nc.vector.tensor_add(ot[:, 0:OH], ps0[:], lgt[:, mt * D : mt * D + OH])
                nc.vector.tensor_add(ot[:, OH:D], ps1[:], lgt[:, mt * D + OH : (mt + 1) * D])
                m0 = (mb * NMT + mt) * 128
                nc.scalar.dma_start(out[m0 : m0 + 128, :], ot[:])

            # Prologue: block 0, all 4 m-tiles k-major across 8 psum banks,
            # so each arriving (xa chunk, W chunk) pair immediately feeds 8
            # matmuls while the rest of W streams in.
            pss = [
                [
                    pspool.tile([128, OH], f32, tag="ps", name=f"ps_{mt}_{h}")
                    for h in range(2)
                ]
                for mt in range(NMT)
            ]
            for k in range(NK):
                for mt in range(NMT):
                    for h in range(2):
                        nc.tensor.matmul(
                            pss[mt][h][:],
                            xsl(xt0, k, mt),
                            wts[k][:, h * OH : (h + 1) * OH],
                            start=(k == 0),
                            stop=(k == NK - 1),
                        )
            for mt in range(NMT):
                finish(0, mt, pss[mt][0], pss[mt][1], lgt0)

            # Steady state: one m-tile at a time, k outer / half inner so
            # consecutive matmuls share the stationary x chunk.
            for mb in range(1, NMB):
                xtn = xpool.tile([128, NK * MB], bf16, tag="x")
                nc.sync.dma_start(xtn[:], xa[:, mb * NK * MB : (mb + 1) * NK * MB])
                xts[mb] = xtn
                lgtn = lgpool.tile([128, NMT * D], bf16, tag="lg")
                nc.sync.dma_start(lgtn[:], lga[:, mb * NMT * D : (mb + 1) * NMT * D])
                lgts[mb] = lgtn
                xt = xts[mb]
                for mt in range(NMT):
                    ps0 = pspool.tile([128, OH], f32, tag="ps")
                    ps1 = pspool.tile([128, OH], f32, tag="ps")
                    for k in range(NK):
                        nc.tensor.matmul(
                            ps0[:], xsl(xt, k, mt), wts[k][:, 0:OH],
                            start=(k == 0), stop=(k == NK - 1),
                        )
                        nc.tensor.matmul(
                            ps1[:], xsl(xt, k, mt), wts[k][:, OH:D],
                            start=(k == 0), stop=(k == NK - 1),
                        )
                    finish(mb, mt, ps0, ps1, lgts[mb])

    nc.compile()
    return nc


def _prepare(x, W, A_table, B_table, domain_id):
    import ml_dtypes

    bf16 = np.dtype(ml_dtypes.bfloat16)
    x = np.asarray(x, dtype=np.float32)
    W = np.asarray(W, dtype=np.float32)
    A = np.asarray(A_table, dtype=np.float64)
    Bt = np.asarray(B_table, dtype=np.float64)
    dom = np.asarray(domain_id).astype(np.int64)

    sA = A.reshape(ND, R, D).sum(axis=2)                        # [ND, R]
    L = np.einsum("dr,dro->do", sA, Bt.reshape(ND, R, D))       # [ND, D]
    Lg = L.astype(np.float32)[dom].astype(bf16)                 # [B, D]

    wa = np.ascontiguousarray(W.T.astype(bf16))                 # [D, D]
    xT = np.ascontiguousarray(x.T).astype(bf16)                 # [D, B]

    in_maps = []
    for c in range(N_CORES):
        sl = slice(c * BS, (c + 1) * BS)
        # chunk-major: xa[p, mb, k, j] = xT[k*128 + p, c*BS + mb*MB + j]
        xa_c = np.ascontiguousarray(
            xT[:, sl].reshape(NK, 128, NMB, MB).transpose(1, 2, 0, 3)
        ).reshape(128, NMB * NK * MB)
        # lga[p, mb, mt, o] = Lg[c*BS + (mb*4+mt)*128 + p, o]
        lga_c = np.ascontiguousarray(
            Lg[sl].reshape(NMB, NMT, 128, D).transpose(2, 0, 1, 3)
        ).reshape(128, NMB * NMT * D)
        in_maps.append({"xa": xa_c, "wa": wa, "lga": lga_c})
    return in_maps


def kernel(x, W, A_table, B_table, domain_id, _trace=False):
    in_maps = _prepare(x, W, A_table, B_table, domain_id)
    nc = _build()
    res = bass_utils.run_bass_kernel_spmd(
        nc, in_maps, core_ids=list(range(N_CORES)), trace=_trace
    )
    out = np.concatenate(
        [np.asarray(res.results[c]["out"]).astype(np.float32) for c in range(N_CORES)],
        axis=0,
    )
    if _trace:
        kernel.last_results = res
    return out
